# revision 30
# baseline (speedup 1.0000x reference)
"""Binary RNN (KernelBRNN) Trainium2 kernel, v2.

Math: 1024 sequential steps; each step runs 7 binary layers
  x <- sign(x @ W_l - t_l)  with x in {+-1}^[B,512], W in {+-1}^[512,512],
then logits_t = x[:, 384:] @ head.

Mapping onto TRN2 (per core, batch sharded B=128 -> b=16):
- State kept feature-major in SBUF: S[128 part = feature%128, 4 chunks * b cols],
  ping-ponged between two buffers per layer parity.
- Mixed sign domains so the thresholding splits across TWO engines that run in
  parallel with the PE:
    chunks 0,1 -> DVE tensor_scalar (psum >= t) - 0.5 in {+-0.5}; the consuming
      weight tiles are scaled x2 (fp8 +-2, exact), so no offset correction.
    chunks 2,3 -> ACT activation Sign(psum + (0.25 - t)) in {+-1}; consuming
      weight tiles are +-1. psum is integer-valued so the 0.25 nudge makes
      Sign() implement >= exactly.
- Each layer: 16 weight-stationary matmuls out[m] += W[l,k,m].T @ S[k],
  one PSUM bank per m-group (4 tags x bufs=2 = 8 banks).
- The embedding slice (chunk 3 of each step's layer-0 input, {+-1}) is
  precomputed on host for all t and read directly by the layer-0 k=3 matmuls
  (no per-step copy).
- Layer-6 m=3 output (the read slice, {+-1}) is signed into a [128, T*b] SBUF
  history buffer; after the time loop one batched GEMM with head (+-1) produces
  exact integer logits in f32.
- Time loop unrolled UNROLL steps per For_i iteration to amortize the
  hardware-loop branch on each engine queue.
"""

import sys
import numpy as np

sys.path.insert(0, "/opt/trn_rl_repo")

import ml_dtypes  # noqa: E402
from contextlib import ExitStack  # noqa: E402

import concourse.bass as bass  # noqa: E402
import concourse.mybir as mybir  # noqa: E402
import concourse.tile as tile  # noqa: E402
from concourse import bacc  # noqa: E402
from concourse import bass_utils  # noqa: E402
from concourse.bass import ds  # noqa: E402

CARRY = 384
READ = 128
D = 512
VOCAB = 128
L = 7
B = 128
T_FULL = 1024
NCORES = 8
PB = B // NCORES  # per-core batch = 16
UNROLL = 4

FP8 = mybir.dt.float8e4
F32 = mybir.dt.float32
NP_FP8 = ml_dtypes.float8_e4m3

AluOp = mybir.AluOpType
ActFn = mybir.ActivationFunctionType

# chunk m sign domain: True -> DVE {+-0.5}, False -> ACT {+-1}
DVE_CHUNK = [True, False, False, True]

# Per-layer MM issue order (annealed against a steady-state latency model of
# PE issue/drain + sem + sign-engine occupancy; ~23% lower layer period than
# the naive m-outer k-inner order).
MM_ORDER = [
    (1, 0), (1, 1), (0, 0), (3, 1), (2, 0), (0, 1), (0, 3), (1, 3),
    (1, 2), (0, 2), (2, 2), (3, 3), (3, 2), (2, 1), (2, 3), (3, 0),
]


def build_program(T: int, n_cores: int = NCORES):
    """Build the Bass program (identical on every core; inputs differ)."""
    b = PB
    assert T % UNROLL == 0
    nc = bacc.Bacc(
        "TRN2",
        target_bir_lowering=False,
        debug=False,
        enable_asserts=False,
        num_devices=n_cores,
    )

    wt = nc.dram_tensor("wt", [128, L * 16 * 128], FP8, kind="ExternalInput").ap()
    emb = nc.dram_tensor("emb", [128, T * b], FP8, kind="ExternalInput").ap()
    s0 = nc.dram_tensor("s0", [128, 3 * b], FP8, kind="ExternalInput").ap()
    # thr_dve[:, l*2+m] = t[l, m-chunk] for m in {0,1}
    # thr_act[:, l*2+(m-2)] = 0.25 - t[l, m-chunk] for m in {2,3}
    thrd = nc.dram_tensor("thrd", [128, L * 2], F32, kind="ExternalInput").ap()
    thra = nc.dram_tensor("thra", [128, L * 2], F32, kind="ExternalInput").ap()
    hd = nc.dram_tensor("hd", [128, VOCAB], FP8, kind="ExternalInput").ap()
    # vocab-major: col r >= pad holds the logits of
    # (step, lane) = divmod(r - pad, b); host transposes to [b,T,V].
    PAD = UNROLL * b
    out = nc.dram_tensor(
        "logits", [VOCAB, T * b + PAD], F32, kind="ExternalOutput"
    ).ap()

    with tile.TileContext(nc) as tc, ExitStack() as ctx:
        pers = ctx.enter_context(tc.tile_pool(name="pers", bufs=1))
        w_s = pers.tile([128, L * 16 * 128], FP8, tag="w_s", name="w_s")
        e_s = pers.tile([128, T * b], FP8, tag="e_s", name="e_s")
        rb = pers.tile([128, T * b + PAD], FP8, tag="rb", name="rb")
        thrd_s = pers.tile([128, L * 2], F32, tag="thrd_s", name="thrd_s")
        thra_s = pers.tile([128, L * 2], F32, tag="thra_s", name="thra_s")
        hd_s = pers.tile([128, VOCAB], FP8, tag="hd_s", name="hd_s")
        # rotating state buffers: layer reads st[p], writes st[(p+1)%NST];
        # NST=4 needs UNROLL*L % NST == 0 for a loop-consistent static body.
        NST = 4
        assert (UNROLL * L) % NST == 0
        st = [
            pers.tile([128, 4 * b], FP8, tag=f"st{i}", name=f"st{i}")
            for i in range(NST)
        ]

        # small tensors + the first few steps' embeddings first, so the loop
        # can start while the bulk of e_s still streams in.
        e_head = min(64 * b, T * b)
        nc.sync.dma_start(thrd_s[:], thrd)
        nc.sync.dma_start(thra_s[:], thra)
        nc.sync.dma_start(hd_s[:], hd)
        nc.sync.dma_start(st[0][:, 0 : 3 * b], s0)
        nc.sync.dma_start(e_s[:, 0:e_head], emb[:, 0:e_head])
        nc.sync.dma_start(w_s[:], wt)
        if e_head < T * b:
            nc.sync.dma_start(e_s[:, e_head:], emb[:, e_head:])

        rec_psum = ExitStack()
        psum = rec_psum.enter_context(tc.tile_pool(name="psum", bufs=2, space="PSUM"))

        # Dummy Sign activation before the loop so the act-table fixpoint sees
        # the 'small' table loaded on the loop preheader path and hoists the
        # per-iteration ACT_TABLE_LOAD out of the loop.
        warm = pers.tile([128, 1], FP8, tag="warm", name="warm")
        nc.scalar.activation(warm[:], thra_s[:, 0:1], ActFn.Sign, bias=0.0, scale=1.0)

        # first/last issue position of each group within MM_ORDER
        first_pos = {m: min(i for i, (mm, _) in enumerate(MM_ORDER) if mm == m)
                     for m in range(4)}
        last_pos = {m: max(i for i, (mm, _) in enumerate(MM_ORDER) if mm == m)
                    for m in range(4)}

        def emit_step(t_ap_base, u):
            """Emit one timestep; t_ap_base is the For_i var, u the unroll idx."""
            # rotation index of the layer-0 input buffer for this step
            par0 = (u * L) % NST
            for layer in range(L):
                cur = st[(par0 + layer) % NST]
                nxt = st[(par0 + layer + 1) % NST]
                tiles = {
                    m: psum.tile([128, b], F32, tag=f"ps{m}", bufs=2,
                                 name=f"ps{m}")
                    for m in range(4)
                }

                def emit_sign(m):
                    ps = tiles[m]
                    if layer == L - 1 and m == 3:
                        dst = rb[:, ds(t_ap_base * (UNROLL * b) + (u * b + PAD), b)]
                    else:
                        dst = nxt[:, m * b : (m + 1) * b]
                    if DVE_CHUNK[m]:
                        c = layer * 2 + (0 if m == 0 else 1)
                        nc.vector.tensor_scalar(
                            dst,
                            ps[:],
                            thrd_s[:, c : c + 1],
                            0.5,
                            AluOp.is_ge,
                            AluOp.subtract,
                        )
                    else:
                        c = layer * 2 + (0 if m == 1 else 1)
                        nc.scalar.activation(
                            dst,
                            ps[:],
                            ActFn.Sign,
                            bias=thra_s[:, c : c + 1],
                            scale=1.0,
                        )

                for pos, (m, k) in enumerate(MM_ORDER):
                    col = ((layer * 4 + k) * 4 + m) * 128
                    if layer == 0 and k == 3:
                        rhs = e_s[:, ds(t_ap_base * (UNROLL * b) + u * b, b)]
                    else:
                        rhs = cur[:, k * b : (k + 1) * b]
                    nc.tensor.matmul(
                        tiles[m][:],
                        w_s[:, col : col + 128],
                        rhs,
                        start=(pos == first_pos[m]),
                        stop=(pos == last_pos[m]),
                    )
                    if pos == last_pos[m]:
                        emit_sign(m)

        # deep ring: the DVE copy must never wait on an in-flight out-DMA
        # (DVE is in-order — a stalled copy blocks every sign op behind it)
        with tc.For_i(0, T // UNROLL, 1) as t:
            for u in range(UNROLL):
                emit_step(t, u)

        rec_psum.close()

        # head GEMM over the full read history, post-loop: hd_s stationary,
        # rb streams in 512-col blocks -> vocab-major [VOCAB, 512] outputs,
        # contiguous 256KB DMAs.
        HB = min(512, T * b)
        with tc.tile_pool(name="hpsum", bufs=4, space="PSUM") as hpsum, tc.tile_pool(
            name="hout", bufs=4
        ) as hout:
            n_blk = T * b // HB
            for j in range(n_blk):
                c0 = PAD + j * HB
                hp = hpsum.tile([VOCAB, HB], F32, tag="h", name="hp")
                nc.tensor.matmul(
                    hp[:], hd_s[:], rb[:, c0 : c0 + HB], start=True, stop=True
                )
                ho = hout.tile([VOCAB, HB], F32, tag="o", name="ho")
                nc.vector.tensor_copy(ho[:], hp[:])
                nc.sync.dma_start(out[:, c0 : c0 + HB], ho[:])

    nc.compile()
    return nc


def prep_inputs(tokens, initial, embed, ff, ff_thresh, head, T: int):
    """Host-side packing -> list of per-core input dicts."""
    tokens = np.asarray(tokens)
    initial = np.asarray(initial, dtype=np.float32)
    embed = np.asarray(embed, dtype=np.float32)
    ff = np.asarray(ff, dtype=np.float32)
    ff_thresh = np.asarray(ff_thresh, dtype=np.float32)
    head = np.asarray(head, dtype=np.float32)

    b = PB
    # weight tiles: col block ((l*4+k)*4+m)*128 holds ff[l, 128k:128k+128, 128m:..]
    # scaled x2 for k-chunks whose source is a DVE ({+-0.5}) chunk.
    wt = np.empty((128, L * 16 * 128), dtype=NP_FP8)
    for layer in range(L):
        for k in range(4):
            # source chunk k domain: {+-0.5} (DVE) -> x2, {+-1} (ACT/embed) -> x1
            scale = 2.0 if DVE_CHUNK[k] else 1.0
            for m in range(4):
                col = ((layer * 4 + k) * 4 + m) * 128
                wt[:, col : col + 128] = (
                    scale * ff[layer, 128 * k : 128 * (k + 1), 128 * m : 128 * (m + 1)]
                ).astype(NP_FP8)

    thrd = np.empty((128, L * 2), dtype=np.float32)
    thra = np.empty((128, L * 2), dtype=np.float32)
    dve_chunks = [m for m in range(4) if DVE_CHUNK[m]]
    act_chunks = [m for m in range(4) if not DVE_CHUNK[m]]
    for layer in range(L):
        for i, m in enumerate(dve_chunks):
            thrd[:, layer * 2 + i] = ff_thresh[layer, 128 * m : 128 * (m + 1)]
        for i, m in enumerate(act_chunks):
            thra[:, layer * 2 + i] = 0.25 - ff_thresh[
                layer, 128 * m : 128 * (m + 1)
            ]

    s0 = np.empty((128, 3 * b), dtype=NP_FP8)
    for m in range(3):
        scale = 0.5 if DVE_CHUNK[m] else 1.0
        s0[:, m * b : (m + 1) * b] = np.repeat(
            (scale * initial[128 * m : 128 * (m + 1)])[:, None], b, axis=1
        ).astype(NP_FP8)

    # rb stores chunk-3 domain values; head scaled to make logits exact ints
    hd_scale = 2.0 if DVE_CHUNK[3] else 1.0
    hd = (hd_scale * head).astype(NP_FP8)  # [128 read-feature, VOCAB]

    in_maps = []
    for c in range(NCORES):
        tok_c = tokens[c * b : (c + 1) * b, :T]  # [b, T]
        g = embed[tok_c]  # [b, T, 128]
        e = np.ascontiguousarray(g.transpose(2, 1, 0)).reshape(128, T * b)
        # embed feeds chunk 3: match its domain
        e = ((0.5 if DVE_CHUNK[3] else 1.0) * e).astype(NP_FP8)
        in_maps.append(
            {"wt": wt, "emb": e, "s0": s0, "thrd": thrd, "thra": thra, "hd": hd}
        )
    return in_maps


_CACHE = {}


def _get_program(T: int):
    if T not in _CACHE:
        _CACHE[T] = build_program(T)
    return _CACHE[T]


def run_on_hw(inputs: dict, T: int = T_FULL, trace: bool = False):
    nc = _get_program(T)
    in_maps = prep_inputs(
        inputs["tokens"],
        inputs["initial"],
        inputs["embed"],
        inputs["ff"],
        inputs["ff_thresh"],
        inputs["head"],
        T,
    )
    res = bass_utils.run_bass_kernel_spmd(
        nc, in_maps, core_ids=list(range(NCORES)), trace=trace
    )
    pad = UNROLL * PB
    outs = [
        np.ascontiguousarray(
            r["logits"][:, pad:].T.reshape(T, PB, VOCAB).transpose(1, 0, 2)
        )
        for r in res.results
    ]
    full = np.concatenate(outs, axis=0)  # [B, T, VOCAB] f32
    return full, res


def kernel(**inputs) -> np.ndarray:
    out, _ = run_on_hw(inputs, T=T_FULL, trace=False)
    return out


# revision 31
# speedup vs baseline: 1.1896x; 1.1896x over previous
"""Binary RNN (KernelBRNN) Trainium2 kernel, v2.

Math: 1024 sequential steps; each step runs 7 binary layers
  x <- sign(x @ W_l - t_l)  with x in {+-1}^[B,512], W in {+-1}^[512,512],
then logits_t = x[:, 384:] @ head.

Mapping onto TRN2 (per core, batch sharded B=128 -> b=16):
- State kept feature-major in SBUF: S[128 part = feature%128, 4 chunks * b cols],
  ping-ponged between two buffers per layer parity.
- Mixed sign domains so the thresholding splits across TWO engines that run in
  parallel with the PE:
    chunks 0,1 -> DVE tensor_scalar (psum >= t) - 0.5 in {+-0.5}; the consuming
      weight tiles are scaled x2 (fp8 +-2, exact), so no offset correction.
    chunks 2,3 -> ACT activation Sign(psum + (0.25 - t)) in {+-1}; consuming
      weight tiles are +-1. psum is integer-valued so the 0.25 nudge makes
      Sign() implement >= exactly.
- Each layer: 16 weight-stationary matmuls out[m] += W[l,k,m].T @ S[k],
  one PSUM bank per m-group (4 tags x bufs=2 = 8 banks).
- The embedding slice (chunk 3 of each step's layer-0 input, {+-1}) is
  precomputed on host for all t and read directly by the layer-0 k=3 matmuls
  (no per-step copy).
- Layer-6 m=3 output (the read slice, {+-1}) is signed into a [128, T*b] SBUF
  history buffer; after the time loop one batched GEMM with head (+-1) produces
  exact integer logits in f32.
- Time loop unrolled UNROLL steps per For_i iteration to amortize the
  hardware-loop branch on each engine queue.
"""

import sys
import numpy as np

sys.path.insert(0, "/opt/trn_rl_repo")

import ml_dtypes  # noqa: E402
from contextlib import ExitStack  # noqa: E402

import concourse.bass as bass  # noqa: E402
import concourse.mybir as mybir  # noqa: E402
import concourse.tile as tile  # noqa: E402
from concourse import bacc  # noqa: E402
from concourse import bass_utils  # noqa: E402
from concourse.bass import ds  # noqa: E402

CARRY = 384
READ = 128
D = 512
VOCAB = 128
L = 7
B = 128
T_FULL = 1024
NCORES = 8
PB = B // NCORES  # per-core batch = 16
UNROLL = 4

FP8 = mybir.dt.float8e4
F32 = mybir.dt.float32
NP_FP8 = ml_dtypes.float8_e4m3

AluOp = mybir.AluOpType
ActFn = mybir.ActivationFunctionType

# chunk m sign domain: True -> DVE {+-0.5}, False -> ACT {+-1}
DVE_CHUNK = [True, False, False, True]

# Per-layer MM issue order (annealed against a steady-state latency model of
# PE issue/drain + sem + sign-engine occupancy; ~23% lower layer period than
# the naive m-outer k-inner order).
MM_ORDER = [
    (1, 0), (1, 1), (0, 0), (3, 1), (2, 0), (0, 1), (0, 3), (1, 3),
    (1, 2), (0, 2), (2, 2), (3, 3), (3, 2), (2, 1), (2, 3), (3, 0),
]


def build_program(T: int, n_cores: int = NCORES):
    """Build the Bass program (identical on every core; inputs differ)."""
    b = PB
    assert T % UNROLL == 0
    nc = bacc.Bacc(
        "TRN2",
        target_bir_lowering=False,
        debug=False,
        enable_asserts=False,
        num_devices=n_cores,
    )

    wt = nc.dram_tensor("wt", [128, L * 16 * 128], FP8, kind="ExternalInput").ap()
    emb = nc.dram_tensor("emb", [128, T * b], FP8, kind="ExternalInput").ap()
    s0 = nc.dram_tensor("s0", [128, 3 * b], FP8, kind="ExternalInput").ap()
    # thr_dve[:, l*2+m] = t[l, m-chunk] for m in {0,1}
    # thr_act[:, l*2+(m-2)] = 0.25 - t[l, m-chunk] for m in {2,3}
    thrd = nc.dram_tensor("thrd", [128, L * 2], F32, kind="ExternalInput").ap()
    thra = nc.dram_tensor("thra", [128, L * 2], F32, kind="ExternalInput").ap()
    hd = nc.dram_tensor("hd", [128, VOCAB], FP8, kind="ExternalInput").ap()
    out = nc.dram_tensor("logits", [b, T, VOCAB], F32, kind="ExternalOutput").ap()

    with tile.TileContext(nc) as tc, ExitStack() as ctx:
        pers = ctx.enter_context(tc.tile_pool(name="pers", bufs=1))
        w_s = pers.tile([128, L * 16 * 128], FP8, tag="w_s", name="w_s")
        e_s = pers.tile([128, T * b], FP8, tag="e_s", name="e_s")
        rb = pers.tile([128, T * b], FP8, tag="rb", name="rb")
        thrd_s = pers.tile([128, L * 2], F32, tag="thrd_s", name="thrd_s")
        thra_s = pers.tile([128, L * 2], F32, tag="thra_s", name="thra_s")
        hd_s = pers.tile([128, VOCAB], FP8, tag="hd_s", name="hd_s")
        # rotating state buffers: layer reads st[p], writes st[(p+1)%NST];
        # NST=4 needs UNROLL*L % NST == 0 for a loop-consistent static body.
        NST = 4
        assert (UNROLL * L) % NST == 0
        st = [
            pers.tile([128, 4 * b], FP8, tag=f"st{i}", name=f"st{i}")
            for i in range(NST)
        ]

        nc.sync.dma_start(w_s[:], wt)
        nc.sync.dma_start(e_s[:], emb)
        nc.sync.dma_start(thrd_s[:], thrd)
        nc.sync.dma_start(thra_s[:], thra)
        nc.sync.dma_start(hd_s[:], hd)
        nc.sync.dma_start(st[0][:, 0 : 3 * b], s0)

        rec_psum = ExitStack()
        psum = rec_psum.enter_context(tc.tile_pool(name="psum", bufs=2, space="PSUM"))

        # Dummy Sign activation before the loop so the act-table fixpoint sees
        # the 'small' table loaded on the loop preheader path and hoists the
        # per-iteration ACT_TABLE_LOAD out of the loop.
        warm = pers.tile([128, 1], FP8, tag="warm", name="warm")
        nc.scalar.activation(warm[:], thra_s[:, 0:1], ActFn.Sign, bias=0.0, scale=1.0)

        # first/last issue position of each group within MM_ORDER
        first_pos = {m: min(i for i, (mm, _) in enumerate(MM_ORDER) if mm == m)
                     for m in range(4)}
        last_pos = {m: max(i for i, (mm, _) in enumerate(MM_ORDER) if mm == m)
                    for m in range(4)}

        def emit_step(t_ap_base, u):
            """Emit one timestep; t_ap_base is the For_i var, u the unroll idx."""
            # rotation index of the layer-0 input buffer for this step
            par0 = (u * L) % NST
            for layer in range(L):
                cur = st[(par0 + layer) % NST]
                nxt = st[(par0 + layer + 1) % NST]
                tiles = {
                    m: psum.tile([128, b], F32, tag=f"ps{m}", bufs=2,
                                 name=f"ps{m}")
                    for m in range(4)
                }

                def emit_sign(m):
                    ps = tiles[m]
                    if layer == L - 1 and m == 3:
                        dst = rb[:, ds(t_ap_base * (UNROLL * b) + u * b, b)]
                    else:
                        dst = nxt[:, m * b : (m + 1) * b]
                    if DVE_CHUNK[m]:
                        c = layer * 2 + (0 if m == 0 else 1)
                        nc.vector.tensor_scalar(
                            dst,
                            ps[:],
                            thrd_s[:, c : c + 1],
                            0.5,
                            AluOp.is_ge,
                            AluOp.subtract,
                        )
                    else:
                        c = layer * 2 + (0 if m == 1 else 1)
                        nc.scalar.activation(
                            dst,
                            ps[:],
                            ActFn.Sign,
                            bias=thra_s[:, c : c + 1],
                            scale=1.0,
                        )

                for pos, (m, k) in enumerate(MM_ORDER):
                    col = ((layer * 4 + k) * 4 + m) * 128
                    if layer == 0 and k == 3:
                        rhs = e_s[:, ds(t_ap_base * (UNROLL * b) + u * b, b)]
                    else:
                        rhs = cur[:, k * b : (k + 1) * b]
                    nc.tensor.matmul(
                        tiles[m][:],
                        w_s[:, col : col + 128],
                        rhs,
                        start=(pos == first_pos[m]),
                        stop=(pos == last_pos[m]),
                    )
                    if pos == last_pos[m]:
                        emit_sign(m)

        with tc.For_i(0, T // UNROLL, 1) as t:
            for u in range(UNROLL):
                emit_step(t, u)

        rec_psum.close()

        # head GEMM over the full read history: logits[(t,i), v]
        out_t = out.rearrange("i t v -> t i v")
        with tc.tile_pool(name="hpsum", bufs=1, space="PSUM") as hpsum, tc.tile_pool(
            name="hout", bufs=1
        ) as hout:
            n_blk = T * b // 128
            for j in range(n_blk):
                ps = hpsum.tile([128, VOCAB], F32, tag=f"h{j % 4}")
                nc.tensor.matmul(
                    ps[:],
                    rb[:, j * 128 : (j + 1) * 128],
                    hd_s[:],
                    start=True,
                    stop=True,
                )
                ot = hout.tile([128, VOCAB], F32, tag=f"o{j % 4}")
                nc.vector.tensor_copy(ot[:], ps[:])
                tpb = 128 // b  # timesteps per 128-row block
                nc.sync.dma_start(out_t[j * tpb : (j + 1) * tpb], ot[:])

    nc.compile()
    return nc


def prep_inputs(tokens, initial, embed, ff, ff_thresh, head, T: int):
    """Host-side packing -> list of per-core input dicts."""
    tokens = np.asarray(tokens)
    initial = np.asarray(initial, dtype=np.float32)
    embed = np.asarray(embed, dtype=np.float32)
    ff = np.asarray(ff, dtype=np.float32)
    ff_thresh = np.asarray(ff_thresh, dtype=np.float32)
    head = np.asarray(head, dtype=np.float32)

    b = PB
    # weight tiles: col block ((l*4+k)*4+m)*128 holds ff[l, 128k:128k+128, 128m:..]
    # scaled x2 for k-chunks whose source is a DVE ({+-0.5}) chunk.
    wt = np.empty((128, L * 16 * 128), dtype=NP_FP8)
    for layer in range(L):
        for k in range(4):
            # source chunk k domain: {+-0.5} (DVE) -> x2, {+-1} (ACT/embed) -> x1
            scale = 2.0 if DVE_CHUNK[k] else 1.0
            for m in range(4):
                col = ((layer * 4 + k) * 4 + m) * 128
                wt[:, col : col + 128] = (
                    scale * ff[layer, 128 * k : 128 * (k + 1), 128 * m : 128 * (m + 1)]
                ).astype(NP_FP8)

    thrd = np.empty((128, L * 2), dtype=np.float32)
    thra = np.empty((128, L * 2), dtype=np.float32)
    dve_chunks = [m for m in range(4) if DVE_CHUNK[m]]
    act_chunks = [m for m in range(4) if not DVE_CHUNK[m]]
    for layer in range(L):
        for i, m in enumerate(dve_chunks):
            thrd[:, layer * 2 + i] = ff_thresh[layer, 128 * m : 128 * (m + 1)]
        for i, m in enumerate(act_chunks):
            thra[:, layer * 2 + i] = 0.25 - ff_thresh[
                layer, 128 * m : 128 * (m + 1)
            ]

    s0 = np.empty((128, 3 * b), dtype=NP_FP8)
    for m in range(3):
        scale = 0.5 if DVE_CHUNK[m] else 1.0
        s0[:, m * b : (m + 1) * b] = np.repeat(
            (scale * initial[128 * m : 128 * (m + 1)])[:, None], b, axis=1
        ).astype(NP_FP8)

    # rb stores chunk-3 domain values; head scaled to make logits exact ints
    hd_scale = 2.0 if DVE_CHUNK[3] else 1.0
    hd = (hd_scale * head).astype(NP_FP8)  # [128 read-feature, VOCAB]

    in_maps = []
    for c in range(NCORES):
        tok_c = tokens[c * b : (c + 1) * b, :T]  # [b, T]
        g = embed[tok_c]  # [b, T, 128]
        e = np.ascontiguousarray(g.transpose(2, 1, 0)).reshape(128, T * b)
        # embed feeds chunk 3: match its domain
        e = ((0.5 if DVE_CHUNK[3] else 1.0) * e).astype(NP_FP8)
        in_maps.append(
            {"wt": wt, "emb": e, "s0": s0, "thrd": thrd, "thra": thra, "hd": hd}
        )
    return in_maps


_CACHE = {}


def _get_program(T: int):
    if T not in _CACHE:
        _CACHE[T] = build_program(T)
    return _CACHE[T]


def run_on_hw(inputs: dict, T: int = T_FULL, trace: bool = False):
    nc = _get_program(T)
    in_maps = prep_inputs(
        inputs["tokens"],
        inputs["initial"],
        inputs["embed"],
        inputs["ff"],
        inputs["ff_thresh"],
        inputs["head"],
        T,
    )
    res = bass_utils.run_bass_kernel_spmd(
        nc, in_maps, core_ids=list(range(NCORES)), trace=trace
    )
    outs = [r["logits"] for r in res.results]
    full = np.concatenate(outs, axis=0)  # [B, T, VOCAB] f32
    return full, res


def kernel(**inputs) -> np.ndarray:
    out, _ = run_on_hw(inputs, T=T_FULL, trace=False)
    return out


# revision 34
# speedup vs baseline: 1.2012x; 1.0098x over previous
"""Binary RNN (KernelBRNN) Trainium2 kernel, v2.

Math: 1024 sequential steps; each step runs 7 binary layers
  x <- sign(x @ W_l - t_l)  with x in {+-1}^[B,512], W in {+-1}^[512,512],
then logits_t = x[:, 384:] @ head.

Mapping onto TRN2 (per core, batch sharded B=128 -> b=16):
- State kept feature-major in SBUF: S[128 part = feature%128, 4 chunks * b cols],
  ping-ponged between two buffers per layer parity.
- Mixed sign domains so the thresholding splits across TWO engines that run in
  parallel with the PE:
    chunks 0,1 -> DVE tensor_scalar (psum >= t) - 0.5 in {+-0.5}; the consuming
      weight tiles are scaled x2 (fp8 +-2, exact), so no offset correction.
    chunks 2,3 -> ACT activation Sign(psum + (0.25 - t)) in {+-1}; consuming
      weight tiles are +-1. psum is integer-valued so the 0.25 nudge makes
      Sign() implement >= exactly.
- Each layer: 16 weight-stationary matmuls out[m] += W[l,k,m].T @ S[k],
  one PSUM bank per m-group (4 tags x bufs=2 = 8 banks).
- The embedding slice (chunk 3 of each step's layer-0 input, {+-1}) is
  precomputed on host for all t and read directly by the layer-0 k=3 matmuls
  (no per-step copy).
- Layer-6 m=3 output (the read slice, {+-1}) is signed into a [128, T*b] SBUF
  history buffer; after the time loop one batched GEMM with head (+-1) produces
  exact integer logits in f32.
- Time loop unrolled UNROLL steps per For_i iteration to amortize the
  hardware-loop branch on each engine queue.
"""

import sys
import numpy as np

sys.path.insert(0, "/opt/trn_rl_repo")

import ml_dtypes  # noqa: E402
from contextlib import ExitStack  # noqa: E402

import concourse.bass as bass  # noqa: E402
import concourse.mybir as mybir  # noqa: E402
import concourse.tile as tile  # noqa: E402
from concourse import bacc  # noqa: E402
from concourse import bass_utils  # noqa: E402
from concourse.bass import ds  # noqa: E402

CARRY = 384
READ = 128
D = 512
VOCAB = 128
L = 7
B = 128
T_FULL = 1024
NCORES = 8
PB = B // NCORES  # per-core batch = 16
UNROLL = 4

FP8 = mybir.dt.float8e4
F32 = mybir.dt.float32
NP_FP8 = ml_dtypes.float8_e4m3

AluOp = mybir.AluOpType
ActFn = mybir.ActivationFunctionType

# chunk m sign domain: True -> DVE {+-0.5}, False -> ACT {+-1}
DVE_CHUNK = [True, False, False, True]

# Per-layer MM issue order (annealed against a steady-state latency model of
# PE issue/drain + sem + sign-engine occupancy; ~23% lower layer period than
# the naive m-outer k-inner order).
MM_ORDER = [
    (1, 0), (1, 1), (0, 0), (3, 1), (2, 0), (0, 1), (0, 3), (1, 3),
    (1, 2), (0, 2), (2, 2), (3, 3), (3, 2), (2, 1), (2, 3), (3, 0),
]


def build_program(T: int, n_cores: int = NCORES):
    """Build the Bass program (identical on every core; inputs differ)."""
    b = PB
    assert T % UNROLL == 0
    nc = bacc.Bacc(
        "TRN2",
        target_bir_lowering=False,
        debug=False,
        enable_asserts=False,
        num_devices=n_cores,
    )

    wt = nc.dram_tensor("wt", [128, L * 16 * 128], FP8, kind="ExternalInput").ap()
    emb = nc.dram_tensor("emb", [128, T * b], FP8, kind="ExternalInput").ap()
    s0 = nc.dram_tensor("s0", [128, 3 * b], FP8, kind="ExternalInput").ap()
    # thr_dve[:, l*2+m] = t[l, m-chunk] for m in {0,1}
    # thr_act[:, l*2+(m-2)] = 0.25 - t[l, m-chunk] for m in {2,3}
    thrd = nc.dram_tensor("thrd", [128, L * 2], F32, kind="ExternalInput").ap()
    thra = nc.dram_tensor("thra", [128, L * 2], F32, kind="ExternalInput").ap()
    hd = nc.dram_tensor("hd", [128, VOCAB], FP8, kind="ExternalInput").ap()
    # vocab-major: col r holds the logits of (step, lane) = divmod(r, b);
    # host transposes to [b, T, V].
    out = nc.dram_tensor("logits", [VOCAB, T * b], F32, kind="ExternalOutput").ap()

    with tile.TileContext(nc) as tc, ExitStack() as ctx:
        pers = ctx.enter_context(tc.tile_pool(name="pers", bufs=1))
        w_s = pers.tile([128, L * 16 * 128], FP8, tag="w_s", name="w_s")
        e_s = pers.tile([128, T * b], FP8, tag="e_s", name="e_s")
        rb = pers.tile([128, T * b], FP8, tag="rb", name="rb")
        thrd_s = pers.tile([128, L * 2], F32, tag="thrd_s", name="thrd_s")
        thra_s = pers.tile([128, L * 2], F32, tag="thra_s", name="thra_s")
        hd_s = pers.tile([128, VOCAB], FP8, tag="hd_s", name="hd_s")
        # rotating state buffers: layer reads st[p], writes st[(p+1)%NST];
        # NST=4 needs UNROLL*L % NST == 0 for a loop-consistent static body.
        NST = 4
        assert (UNROLL * L) % NST == 0
        st = [
            pers.tile([128, 4 * b], FP8, tag=f"st{i}", name=f"st{i}")
            for i in range(NST)
        ]

        nc.sync.dma_start(w_s[:], wt)
        nc.sync.dma_start(e_s[:], emb)
        nc.sync.dma_start(thrd_s[:], thrd)
        nc.sync.dma_start(thra_s[:], thra)
        nc.sync.dma_start(hd_s[:], hd)
        nc.sync.dma_start(st[0][:, 0 : 3 * b], s0)

        rec_psum = ExitStack()
        psum = rec_psum.enter_context(tc.tile_pool(name="psum", bufs=2, space="PSUM"))

        # Dummy Sign activation before the loop so the act-table fixpoint sees
        # the 'small' table loaded on the loop preheader path and hoists the
        # per-iteration ACT_TABLE_LOAD out of the loop.
        warm = pers.tile([128, 1], FP8, tag="warm", name="warm")
        nc.scalar.activation(warm[:], thra_s[:, 0:1], ActFn.Sign, bias=0.0, scale=1.0)

        # first/last issue position of each group within MM_ORDER
        first_pos = {m: min(i for i, (mm, _) in enumerate(MM_ORDER) if mm == m)
                     for m in range(4)}
        last_pos = {m: max(i for i, (mm, _) in enumerate(MM_ORDER) if mm == m)
                    for m in range(4)}

        def emit_step(t_ap_base, u):
            """Emit one timestep; t_ap_base is the For_i var, u the unroll idx."""
            # rotation index of the layer-0 input buffer for this step
            par0 = (u * L) % NST
            for layer in range(L):
                cur = st[(par0 + layer) % NST]
                nxt = st[(par0 + layer + 1) % NST]
                tiles = {
                    m: psum.tile([128, b], F32, tag=f"ps{m}", bufs=2,
                                 name=f"ps{m}")
                    for m in range(4)
                }

                def emit_sign(m):
                    ps = tiles[m]
                    if layer == L - 1 and m == 3:
                        dst = rb[:, ds(t_ap_base * (UNROLL * b) + u * b, b)]
                    else:
                        dst = nxt[:, m * b : (m + 1) * b]
                    if DVE_CHUNK[m]:
                        c = layer * 2 + (0 if m == 0 else 1)
                        nc.vector.tensor_scalar(
                            dst,
                            ps[:],
                            thrd_s[:, c : c + 1],
                            0.5,
                            AluOp.is_ge,
                            AluOp.subtract,
                        )
                    else:
                        c = layer * 2 + (0 if m == 1 else 1)
                        nc.scalar.activation(
                            dst,
                            ps[:],
                            ActFn.Sign,
                            bias=thra_s[:, c : c + 1],
                            scale=1.0,
                        )

                for pos, (m, k) in enumerate(MM_ORDER):
                    col = ((layer * 4 + k) * 4 + m) * 128
                    if layer == 0 and k == 3:
                        rhs = e_s[:, ds(t_ap_base * (UNROLL * b) + u * b, b)]
                    else:
                        rhs = cur[:, k * b : (k + 1) * b]
                    nc.tensor.matmul(
                        tiles[m][:],
                        w_s[:, col : col + 128],
                        rhs,
                        start=(pos == first_pos[m]),
                        stop=(pos == last_pos[m]),
                    )
                    if pos == last_pos[m]:
                        emit_sign(m)

        with tc.For_i(0, T // UNROLL, 1) as t:
            for u in range(UNROLL):
                emit_step(t, u)

        rec_psum.close()

        # head GEMM over the full read history, vocab-major: hd_s stationary,
        # rb streams in 512-col blocks, contiguous 256KB output DMAs.
        HB = min(512, T * b)
        with tc.tile_pool(name="hpsum", bufs=4, space="PSUM") as hpsum, tc.tile_pool(
            name="hout", bufs=4
        ) as hout:
            n_blk = T * b // HB
            for j in range(n_blk):
                c0 = j * HB
                hp = hpsum.tile([VOCAB, HB], F32, tag="h", name="hp")
                nc.tensor.matmul(
                    hp[:], hd_s[:], rb[:, c0 : c0 + HB], start=True, stop=True
                )
                ho = hout.tile([VOCAB, HB], F32, tag="o", name="ho")
                nc.vector.tensor_copy(ho[:], hp[:])
                nc.sync.dma_start(out[:, c0 : c0 + HB], ho[:])

    nc.compile()
    return nc


def prep_inputs(tokens, initial, embed, ff, ff_thresh, head, T: int):
    """Host-side packing -> list of per-core input dicts."""
    tokens = np.asarray(tokens)
    initial = np.asarray(initial, dtype=np.float32)
    embed = np.asarray(embed, dtype=np.float32)
    ff = np.asarray(ff, dtype=np.float32)
    ff_thresh = np.asarray(ff_thresh, dtype=np.float32)
    head = np.asarray(head, dtype=np.float32)

    b = PB
    # weight tiles: col block ((l*4+k)*4+m)*128 holds ff[l, 128k:128k+128, 128m:..]
    # scaled x2 for k-chunks whose source is a DVE ({+-0.5}) chunk.
    wt = np.empty((128, L * 16 * 128), dtype=NP_FP8)
    for layer in range(L):
        for k in range(4):
            # source chunk k domain: {+-0.5} (DVE) -> x2, {+-1} (ACT/embed) -> x1
            scale = 2.0 if DVE_CHUNK[k] else 1.0
            for m in range(4):
                col = ((layer * 4 + k) * 4 + m) * 128
                wt[:, col : col + 128] = (
                    scale * ff[layer, 128 * k : 128 * (k + 1), 128 * m : 128 * (m + 1)]
                ).astype(NP_FP8)

    thrd = np.empty((128, L * 2), dtype=np.float32)
    thra = np.empty((128, L * 2), dtype=np.float32)
    dve_chunks = [m for m in range(4) if DVE_CHUNK[m]]
    act_chunks = [m for m in range(4) if not DVE_CHUNK[m]]
    for layer in range(L):
        for i, m in enumerate(dve_chunks):
            thrd[:, layer * 2 + i] = ff_thresh[layer, 128 * m : 128 * (m + 1)]
        for i, m in enumerate(act_chunks):
            thra[:, layer * 2 + i] = 0.25 - ff_thresh[
                layer, 128 * m : 128 * (m + 1)
            ]

    s0 = np.empty((128, 3 * b), dtype=NP_FP8)
    for m in range(3):
        scale = 0.5 if DVE_CHUNK[m] else 1.0
        s0[:, m * b : (m + 1) * b] = np.repeat(
            (scale * initial[128 * m : 128 * (m + 1)])[:, None], b, axis=1
        ).astype(NP_FP8)

    # rb stores chunk-3 domain values; head scaled to make logits exact ints
    hd_scale = 2.0 if DVE_CHUNK[3] else 1.0
    hd = (hd_scale * head).astype(NP_FP8)  # [128 read-feature, VOCAB]

    in_maps = []
    for c in range(NCORES):
        tok_c = tokens[c * b : (c + 1) * b, :T]  # [b, T]
        g = embed[tok_c]  # [b, T, 128]
        e = np.ascontiguousarray(g.transpose(2, 1, 0)).reshape(128, T * b)
        # embed feeds chunk 3: match its domain
        e = ((0.5 if DVE_CHUNK[3] else 1.0) * e).astype(NP_FP8)
        in_maps.append(
            {"wt": wt, "emb": e, "s0": s0, "thrd": thrd, "thra": thra, "hd": hd}
        )
    return in_maps


_CACHE = {}


def _get_program(T: int):
    if T not in _CACHE:
        _CACHE[T] = build_program(T)
    return _CACHE[T]


def run_on_hw(inputs: dict, T: int = T_FULL, trace: bool = False):
    nc = _get_program(T)
    in_maps = prep_inputs(
        inputs["tokens"],
        inputs["initial"],
        inputs["embed"],
        inputs["ff"],
        inputs["ff_thresh"],
        inputs["head"],
        T,
    )
    res = bass_utils.run_bass_kernel_spmd(
        nc, in_maps, core_ids=list(range(NCORES)), trace=trace
    )
    outs = [
        np.ascontiguousarray(
            r["logits"].T.reshape(T, PB, VOCAB).transpose(1, 0, 2)
        )
        for r in res.results
    ]
    full = np.concatenate(outs, axis=0)  # [B, T, VOCAB] f32
    return full, res


def kernel(**inputs) -> np.ndarray:
    out, _ = run_on_hw(inputs, T=T_FULL, trace=False)
    return out
